# revision 4
# baseline (speedup 1.0000x reference)
"""Multi-head attention (B=4, T=S=2048, H=1024, 16 heads x D=64) on 8 TRN2 cores.

Sharding: 2D mesh of batch(4) x head-group(2). Core c = b*2 + g computes, for
its batch b and its 8 heads (ND slice g*512:(g+1)*512):
  - q/k/v projections (bf16 matmuls, fp32 accumulate)
  - attention in transposed [S, T] orientation: scoresT = kT.T @ qT chunks,
    exp on ScalarE (scale folded in), softmax denominator via a ones-column
    appended to v in the AV matmul, normalization by partition-broadcast
    reciprocal
  - partial output projection out_part = ao @ Wo_g.T  ([T, H], fp32)
Host sums the two head-group partials per batch and adds bo.

All matmul inputs bf16 (PSUM accumulates fp32): rel err vs fp32 reference
~3e-3. q/k/v biases are applied in-kernel (they are zero for this problem's
inputs, but supported); bo is added on the host.
"""

import numpy as np
import ml_dtypes

import concourse.bacc as bacc
import concourse.mybir as mybir
import concourse.tile as tile
from concourse.bass_utils import run_bass_kernel_spmd

B, T, H = 4, 2048, 1024
N_HEADS, D = 16, 64
GROUPS = 2
HEADS_PER_GROUP = N_HEADS // GROUPS          # 8
NDG = HEADS_PER_GROUP * D                    # 512
SCALE = 1.0 / float(D) ** 0.5
N_CORES = 8

bf16 = mybir.dt.bfloat16
f32 = mybir.dt.float32
EXP = mybir.ActivationFunctionType.Exp
MULT = mybir.AluOpType.mult
ADD = mybir.AluOpType.add

_CACHED_NC = None


def _build():
    nc = bacc.Bacc("TRN2", target_bir_lowering=False, debug=False)

    xq_d = nc.dram_tensor("xqT", (H, T), bf16, kind="ExternalInput")
    xv_d = nc.dram_tensor("xvT", (H, T), bf16, kind="ExternalInput")
    wq_d = nc.dram_tensor("wqT", (H, NDG), bf16, kind="ExternalInput")
    wk_d = nc.dram_tensor("wkT", (H, NDG), bf16, kind="ExternalInput")
    wv_d = nc.dram_tensor("wvT", (H, NDG), bf16, kind="ExternalInput")
    wo_d = nc.dram_tensor("woT", (NDG, H), bf16, kind="ExternalInput")
    bq_d = nc.dram_tensor("bq", (NDG,), f32, kind="ExternalInput")
    bk_d = nc.dram_tensor("bk", (NDG,), f32, kind="ExternalInput")
    bv_d = nc.dram_tensor("bv", (NDG,), f32, kind="ExternalInput")
    out_d = nc.dram_tensor("outp", (T, H), f32, kind="ExternalOutput")

    with tile.TileContext(nc) as tc:
        with tc.tile_pool(name="w", bufs=1) as wpool, \
             tc.tile_pool(name="data", bufs=1) as dpool, \
             tc.tile_pool(name="exps", bufs=4) as epool, \
             tc.tile_pool(name="norm", bufs=2) as npool, \
             tc.tile_pool(name="stage", bufs=3) as spool, \
             tc.tile_pool(name="ps_sc", bufs=2, space="PSUM") as ps_sc, \
             tc.tile_pool(name="ps_av", bufs=1, space="PSUM") as ps_av, \
             tc.tile_pool(name="ps_pj", bufs=2, space="PSUM") as ps_pj:

            wq_t = wpool.tile([128, 8, NDG], bf16)
            wk_t = wpool.tile([128, 8, NDG], bf16)
            wv_t = wpool.tile([128, 8, NDG], bf16)
            wo_t = wpool.tile([128, 4, H], bf16)
            bq_t = wpool.tile([128, 4], f32)
            bk_t = wpool.tile([128, 4], f32)
            bv_row = wpool.tile([1, NDG], f32)
            bv_bc = wpool.tile([128, NDG], f32)

            xq_t = dpool.tile([128, 8, T], bf16)
            xv_t = dpool.tile([128, 8, T], bf16)
            qT_t = dpool.tile([128, 4, T], bf16)
            kT_t = dpool.tile([128, 4, T], bf16)
            v_t = dpool.tile([128, 16, HEADS_PER_GROUP, D + 1], bf16)
            ao_t = dpool.tile([128, 4, T], bf16)

            # input DMAs, roughly in first-use order
            nc.sync.dma_start(wk_t[:], wk_d.rearrange("(c p) n -> p c n", p=128))
            nc.sync.dma_start(xv_t[:], xv_d.rearrange("(c p) t -> p c t", p=128))
            nc.sync.dma_start(wq_t[:], wq_d.rearrange("(c p) n -> p c n", p=128))
            nc.sync.dma_start(xq_t[:], xq_d.rearrange("(c p) t -> p c t", p=128))
            nc.sync.dma_start(wv_t[:], wv_d.rearrange("(c p) n -> p c n", p=128))
            nc.sync.dma_start(wo_t[:], wo_d.rearrange("(c p) h -> p c h", p=128))
            nc.sync.dma_start(bq_t[:], bq_d.rearrange("(c p) -> p c", p=128))
            nc.sync.dma_start(bk_t[:], bk_d.rearrange("(c p) -> p c", p=128))
            nc.sync.dma_start(bv_row[:], bv_d[None, :])
            nc.gpsimd.partition_broadcast(bv_bc[:], bv_row[0:1, :])
            nc.vector.memset(v_t[:, :, :, D], 1.0)

            def proj_qk(dst_t, w_t, b_t, ndc):
                """q or k projection, ND chunk ndc (128 channels), all T."""
                for t4 in range(4):
                    ps = ps_pj.tile([128, 512], f32, tag="pj")
                    for h in range(8):
                        nc.tensor.matmul(
                            ps[:],
                            w_t[:, h, ndc * 128:(ndc + 1) * 128],
                            xq_t[:, h, t4 * 512:(t4 + 1) * 512] if dst_t is qT_t
                            else xv_t[:, h, t4 * 512:(t4 + 1) * 512],
                            start=(h == 0), stop=(h == 7),
                        )
                    nc.vector.tensor_tensor(
                        dst_t[:, ndc, t4 * 512:(t4 + 1) * 512], ps[:],
                        b_t[:, ndc, None].to_broadcast((128, 512)), ADD)

            def proj_v(half):
                """v projection for ND cols half*256..+256 (heads 4h..4h+3)."""
                for t16 in range(16):
                    ps = ps_pj.tile([128, 512], f32, tag="pj")
                    for h in range(8):
                        nc.tensor.matmul(
                            ps[:, 0:256],
                            xv_t[:, h, t16 * 128:(t16 + 1) * 128],
                            wv_t[:, h, half * 256:(half + 1) * 256],
                            start=(h == 0), stop=(h == 7),
                        )
                    nc.vector.tensor_tensor(
                        v_t[:, t16, 4 * half:4 * half + 4, 0:D],
                        ps[:, 0:256].rearrange("p (hh d) -> p hh d", d=D),
                        bv_bc[:, half * 256:(half + 1) * 256]
                        .rearrange("p (hh d) -> p hh d", d=D), ADD)

            def attn_head(n):
                """Attention for local head n over full T, in 2 blocks of 1024."""
                c, off = n // 2, 64 * (n % 2)
                for tb in range(2):
                    t0 = tb * 1024
                    av = ps_av.tile([128, 1024], f32, tag="av")
                    for s in range(16):
                        sc = ps_sc.tile([128, 1024], f32, tag="sc")
                        for th in range(2):
                            nc.tensor.matmul(
                                sc[:, th * 512:(th + 1) * 512],
                                kT_t[off:off + 64, c, s * 128:(s + 1) * 128],
                                qT_t[off:off + 64, c,
                                     t0 + th * 512:t0 + (th + 1) * 512],
                                start=True, stop=True,
                            )
                        ex = epool.tile([128, 1024], bf16, tag="exp")
                        nc.scalar.activation(ex[:], sc[:], EXP, scale=SCALE)
                        for th in range(2):
                            nc.tensor.matmul(
                                av[0:D + 1, th * 512:(th + 1) * 512],
                                v_t[:, s, n, :],
                                ex[:, th * 512:(th + 1) * 512],
                                start=(s == 0), stop=(s == 15),
                            )
                    # copy AV out of PSUM fast to release the bank, then
                    # normalize asynchronously off the critical path
                    avs = npool.tile([D + 1, 1024], f32, tag="avs")
                    nc.vector.tensor_copy(avs[0:D + 1, :], av[0:D + 1, :])
                    recip = npool.tile([1, 1024], f32, tag="recip")
                    nc.vector.reciprocal(recip[:], avs[D:D + 1, :])
                    bc = npool.tile([64, 1024], f32, tag="bc")
                    nc.gpsimd.partition_broadcast(bc[:], recip[0:1, :])
                    nc.vector.tensor_tensor(
                        ao_t[off:off + 64, c, t0:t0 + 1024],
                        avs[0:D, :], bc[:], MULT)

            # pipelined emission: projections for chunk ndc+1 interleave with
            # attention on the two heads of chunk ndc
            proj_qk(kT_t, wk_t, bk_t, 0)
            proj_qk(qT_t, wq_t, bq_t, 0)
            proj_v(0)
            for ndc in range(4):
                attn_head(2 * ndc)
                if ndc < 3:
                    proj_qk(kT_t, wk_t, bk_t, ndc + 1)
                attn_head(2 * ndc + 1)
                if ndc < 3:
                    proj_qk(qT_t, wq_t, bq_t, ndc + 1)
                if ndc == 1:
                    proj_v(1)

            # output projection: out_part[T, H] = ao @ Wo_g.T
            for t16 in range(16):
                for hh in range(2):
                    ps = ps_pj.tile([128, 512], f32, tag="pj")
                    for nd in range(4):
                        nc.tensor.matmul(
                            ps[:],
                            ao_t[:, nd, t16 * 128:(t16 + 1) * 128],
                            wo_t[:, nd, hh * 512:(hh + 1) * 512],
                            start=(nd == 0), stop=(nd == 3),
                        )
                    st = spool.tile([128, 512], f32, tag="st")
                    nc.vector.tensor_copy(st[:], ps[:])
                    nc.sync.dma_start(
                        out_d[t16 * 128:(t16 + 1) * 128,
                              hh * 512:(hh + 1) * 512], st[:])

    nc.compile()
    return nc


def kernel(**inputs):
    global _CACHED_NC
    query = np.asarray(inputs["query"], dtype=np.float32)
    value = np.asarray(inputs["value"], dtype=np.float32)
    Wq = np.asarray(inputs["Wq"], dtype=np.float32)
    Wk = np.asarray(inputs["Wk"], dtype=np.float32)
    Wv = np.asarray(inputs["Wv"], dtype=np.float32)
    Wo = np.asarray(inputs["Wo"], dtype=np.float32)
    bq = np.asarray(inputs["bq"], dtype=np.float32)
    bk = np.asarray(inputs["bk"], dtype=np.float32)
    bv = np.asarray(inputs["bv"], dtype=np.float32)
    bo = np.asarray(inputs["bo"], dtype=np.float32)

    if _CACHED_NC is None:
        _CACHED_NC = _build()
    nc = _CACHED_NC

    bf = ml_dtypes.bfloat16
    xqT = [np.ascontiguousarray(query[b].T).astype(bf) for b in range(B)]
    xvT = [np.ascontiguousarray(value[b].T).astype(bf) for b in range(B)]
    wqT, wkT, wvT, woT, bqs, bks, bvs = [], [], [], [], [], [], []
    for g in range(GROUPS):
        sl = slice(g * NDG, (g + 1) * NDG)
        wqT.append(np.ascontiguousarray(Wq[sl].T).astype(bf))
        wkT.append(np.ascontiguousarray(Wk[sl].T).astype(bf))
        wvT.append(np.ascontiguousarray(Wv[sl].T).astype(bf))
        woT.append(np.ascontiguousarray(Wo[:, sl].T).astype(bf))
        bqs.append(np.ascontiguousarray(bq[sl]))
        bks.append(np.ascontiguousarray(bk[sl]))
        bvs.append(np.ascontiguousarray(bv[sl]))

    in_maps = []
    for c in range(N_CORES):
        b, g = c // 2, c % 2
        in_maps.append({
            "xqT": xqT[b], "xvT": xvT[b],
            "wqT": wqT[g], "wkT": wkT[g], "wvT": wvT[g], "woT": woT[g],
            "bq": bqs[g], "bk": bks[g], "bv": bvs[g],
        })

    res = run_bass_kernel_spmd(nc, in_maps, core_ids=list(range(N_CORES)))

    out = np.zeros((B, T, H), dtype=np.float32)
    for c in range(N_CORES):
        b = c // 2
        out[b] += res.results[c]["outp"]
    out += bo
    return out
